# revision 28
# baseline (speedup 1.0000x reference)
"""Converse2D-Up (FFT deconvolution upsampler) as a Bass/Tile kernel for TRN2.

Math (validated against the jax reference): the whole pipeline before the
final gelu is linear in x and channel-wise.  With xp = wrap-pad(x) (132x132),
Y = FFT132(xp) = G @ x @ G^T where G = F132 @ P (132x128).  The reference's
264-point spectral transfer function H (built from weight/bias only) is
Hermitian, so out decomposes into 4 polyphase outputs
out_dd = real(IFFT132(Kdd_hat . Y)) with per-channel precomputed spectra
Kdd_hat; the crop leaves exactly 128 rows/cols per phase.  Hermitian symmetry
means only columns v=0..66 of Kdd_hat.Y are needed:
    T1[v,x] = sum_u (Kdd_hat.Y)[u,pv] Ai[x,u]
    out[x,y] = sum_{v} w_v (T1r[v,x] RC[v,y] + T1i[v,x] RS[v,y])

v2: all PE matmuls in bf16 (1 cycle/row vs 4 for fp32), packed N>=256 moving
operands (29 matmuls/image vs 98), the 4 low rows (u=128..131) of the
pointwise complex multiply batched across the 4 images of a channel, and the
gelu+interleave output written with a single contiguous DMA per image.

Sharding: 8 channels per core x 4 batch images; weight/bias-derived spectra
are host-precomputed constants.
"""

import os

import ml_dtypes
import numpy as np

import concourse.bass as bass
import concourse.mybir as mybir
import concourse.tile as tile
from concourse import bacc
from concourse.bass import ts
from concourse.bass_utils import run_bass_kernel_spmd

F32 = mybir.dt.float32
BF16 = mybir.dt.bfloat16
AF = mybir.ActivationFunctionType

SCALE = 2
PAD = 2
EPS = 1e-5
N0 = 128           # input spatial size
NP = N0 + 2 * PAD  # 132 padded
NU = NP * SCALE    # 264 upsampled
NV = NP // 2 + 1   # 67 unique spectral columns
B = 4
C = 64
NCORES = 8
CPC = C // NCORES  # 8 channels per core
NIMG = B * CPC     # 32 images per core

LAST_EXEC_NS = None  # set by kernel() when tracing is enabled


# --------------------------------------------------------------------------
# host-side constant precompute (weight/bias -> per-channel spectra)
# --------------------------------------------------------------------------

def _host_constants(weight, bias):
    w64 = np.asarray(weight, dtype=np.float64)
    b64 = np.asarray(bias, dtype=np.float64)

    # FB = p2o(weight): 264-point OTF of the rolled 3x3 PSF, per channel
    k_h, k_w = w64.shape[-2:]
    otf = np.zeros((C, NU, NU), dtype=np.complex128)
    otf[:, :k_h, :k_w] = w64[0]
    otf = np.roll(otf, (-(k_h // 2), -(k_w // 2)), axis=(-2, -1))
    FB = np.fft.fftn(otf, axes=(-2, -1))                      # (C,264,264)

    biaseps = 1.0 / (1.0 + np.exp(-(b64.reshape(C) - 9.0))) + EPS  # (C,)
    be = biaseps[:, None, None]

    u = np.arange(NU)
    Dr = 1 + np.exp(-2j * np.pi * u / NU)
    D = Dr[:, None] * Dr[None, :]                             # (264,264)

    Gh = np.conj(FB) + be * D[None]
    FBG = FB * Gh

    def quadmean(A):
        return 0.25 * (A[:, :NP, :NP] + A[:, NP:, :NP]
                       + A[:, :NP, NP:] + A[:, NP:, NP:])

    M1 = quadmean(FBG)
    invW = quadmean(np.abs(FB) ** 2)
    M2 = M1 / (invW + be)
    H = (Gh - np.conj(FB) * np.tile(M2, (1, SCALE, SCALE))) / be   # (C,264,264)

    hr = np.fft.ifft2(H, axes=(-2, -1)).real                  # H Hermitian
    # polyphase spectra: Kdd_hat[c,dx,dy] = FFT132(hr[c, dx::2, dy::2])
    kdd = np.empty((C, 2, 2, NP, NV), dtype=np.complex128)
    for dx in range(2):
        for dy in range(2):
            kh = np.fft.fft2(hr[:, dx::2, dy::2], axes=(-2, -1))
            kdd[:, dx, dy] = kh[:, :, :NV]

    # forward matrix G = F132 @ P  (132x128 complex)
    P = np.zeros((NP, N0))
    for m in range(NP):
        P[m, (m - PAD) % N0] = 1.0
    F132 = np.exp(-2j * np.pi * np.outer(np.arange(NP), np.arange(NP)) / NP)
    G = F132 @ P

    # inverse matrix, rows i in [2,130) of iF132/132
    Ai = np.exp(2j * np.pi * np.outer(np.arange(2, 130), np.arange(NP)) / NP) / NP
    Cm, Sm = Ai.real, Ai.imag                                  # (128,132)
    CT, ST = Cm.T, Sm.T                                        # (132,128)

    w_v = np.ones(NV)
    w_v[1:NV - 1] = 2.0
    RC = (Cm[:, :NV] * w_v[None, :]).T                         # (67,128)
    RS = (-Sm[:, :NV] * w_v[None, :]).T

    bf = ml_dtypes.bfloat16

    # per-channel spectra packed for the DVE complex multiply:
    # cols (a, p, v): a in {0,1}; set1=(Kr,Ki), set2=(Ki,Kr)
    kr = np.ascontiguousarray(
        kdd.real.transpose(0, 3, 1, 2, 4)).reshape(C, NP, 4 * NV)
    ki = np.ascontiguousarray(
        kdd.imag.transpose(0, 3, 1, 2, 4)).reshape(C, NP, 4 * NV)
    khi = np.concatenate([kr[:, :128], ki[:, :128],
                          ki[:, :128], kr[:, :128]], axis=2)   # (C,128,1072)
    # lo rows (u=128..131) column-packed: cols = (bi, a, f, v), spectra
    # replicated per batch image so the 4-image cmul is a few wide DVE ops
    klo_s1 = np.concatenate([kr[:, 128:], ki[:, 128:]], axis=2)  # (C,4,536)
    klo_s2 = np.concatenate([ki[:, 128:], kr[:, 128:]], axis=2)
    klo = np.concatenate([np.tile(klo_s1, (1, 1, 4)),
                          np.tile(klo_s2, (1, 1, 4))], axis=2)   # (C,4,4288)

    return {
        "gt264": np.concatenate([G.real.T, G.imag.T], 1).astype(bf),
        "bgt1": np.concatenate([G.real[0:NV].T, G.imag[0:NV].T], 1).astype(bf),
        "bgt2": np.concatenate([-G.imag[0:NV].T, G.real[0:NV].T], 1).astype(bf),
        "cst1": np.concatenate([CT[:128], ST[:128]], 1).astype(bf),
        "cst2": np.concatenate([-ST[:128], CT[:128]], 1).astype(bf),
        "cst1lo": np.concatenate([CT[128:], ST[128:]], 1).astype(bf),
        "cst2lo": np.concatenate([-ST[128:], CT[128:]], 1).astype(bf),
        "rcs": np.concatenate([RC, RS], 1).astype(bf),
        "rcs2": np.concatenate([RS, RC], 1).astype(bf),
        "khi": khi.astype(bf),
        "klo": klo.astype(bf),
    }


# --------------------------------------------------------------------------
# device kernel
# --------------------------------------------------------------------------

def build_nc():
    nc = bacc.Bacc("TRN2", target_bir_lowering=False, debug=False,
                   enable_asserts=False)

    x_t = nc.dram_tensor("x", [NIMG, N0, N0], BF16, kind="ExternalInput")
    khi_t = nc.dram_tensor("khi", [CPC, 128, 1072], BF16, kind="ExternalInput")
    klo_t = nc.dram_tensor("klo", [CPC, 4, 4288], BF16, kind="ExternalInput")
    gt264_t = nc.dram_tensor("gt264", [128, 2 * NP], BF16, kind="ExternalInput")
    bgt1_t = nc.dram_tensor("bgt1", [128, 2 * NV], BF16, kind="ExternalInput")
    bgt2_t = nc.dram_tensor("bgt2", [128, 2 * NV], BF16, kind="ExternalInput")
    cst1_t = nc.dram_tensor("cst1", [128, 256], BF16, kind="ExternalInput")
    cst2_t = nc.dram_tensor("cst2", [128, 256], BF16, kind="ExternalInput")
    cst1lo_t = nc.dram_tensor("cst1lo", [4, 256], BF16, kind="ExternalInput")
    cst2lo_t = nc.dram_tensor("cst2lo", [4, 256], BF16, kind="ExternalInput")
    rcs_t = nc.dram_tensor("rcs", [NV, 256], BF16, kind="ExternalInput")
    rcs2_t = nc.dram_tensor("rcs2", [NV, 256], BF16, kind="ExternalInput")
    out_t = nc.dram_tensor("out", [NIMG, 2 * N0, 2 * N0], F32,
                           kind="ExternalOutput")

    PH4 = 4 * NV          # 268
    with tile.TileContext(nc) as tc:
        with (
            tc.tile_pool(name="consts", bufs=1) as cpool,
            tc.tile_pool(name="kdd", bufs=2) as kpool,
            tc.tile_pool(name="xin", bufs=5) as xpool,
            tc.tile_pool(name="r1", bufs=2) as r1pool,
            tc.tile_pool(name="yev", bufs=2) as ypool,
            tc.tile_pool(name="prod", bufs=2) as papool,
            tc.tile_pool(name="fx", bufs=6) as fxpool,
            tc.tile_pool(name="t1", bufs=2) as t1pool,
            tc.tile_pool(name="osb", bufs=2) as opool,
            tc.tile_pool(name="ppa", bufs=1, space="PSUM") as ppa_pool,
            tc.tile_pool(name="ppy", bufs=1, space="PSUM") as ppy_pool,
            tc.tile_pool(name="pt1", bufs=2, space="PSUM") as pt1_pool,
            tc.tile_pool(name="ppd", bufs=1, space="PSUM") as ppd_pool,
        ):
            gt264 = cpool.tile([128, 2 * NP], BF16)
            nc.sync.dma_start(gt264[:], gt264_t[:])
            bgt1 = cpool.tile([128, 2 * NV], BF16)
            nc.sync.dma_start(bgt1[:], bgt1_t[:])
            bgt2 = cpool.tile([128, 2 * NV], BF16)
            nc.sync.dma_start(bgt2[:], bgt2_t[:])
            cst1 = cpool.tile([128, 256], BF16)
            nc.sync.dma_start(cst1[:], cst1_t[:])
            cst2 = cpool.tile([128, 256], BF16)
            nc.sync.dma_start(cst2[:], cst2_t[:])
            cst1lo = cpool.tile([4, 256], BF16)
            nc.sync.dma_start(cst1lo[:], cst1lo_t[:])
            cst2lo = cpool.tile([4, 256], BF16)
            nc.sync.dma_start(cst2lo[:], cst2lo_t[:])
            rcs = cpool.tile([NV, 256], BF16)
            nc.sync.dma_start(rcs[:], rcs_t[:])
            rcs2 = cpool.tile([NV, 256], BF16)
            nc.sync.dma_start(rcs2[:], rcs2_t[:])

            # -------- software-pipelined slot schedule ----------------
            # slot k emits: C'(k), A(k+5), B(k+4), D(k) on the PE so the
            # psum evictions of image k (scalar/DVE) hide under A/B, and
            # r1(k+5) is ready one full slot before B(k+5) consumes it.
            chan = {}     # c -> (khi, klo)
            xtile = {}    # k -> x tile
            r1s = {}      # k -> r1 tile
            ylops = {}    # c -> ylop tile
            fxs = {}      # k -> (fxr, fxi)
            fxlos = {}    # c -> (fxlr, fxli)

            def emit_kdma(c):
                khi = kpool.tile([128, 1072], BF16, tag="khi")
                nc.sync.dma_start(khi[:], khi_t[c])
                klo = kpool.tile([4, 4288], BF16, tag="klo")
                nc.sync.dma_start(klo[:], klo_t[c])
                chan[c] = (khi, klo)

            def emit_xdma(k):
                xt = xpool.tile([N0, N0], BF16, tag="x")
                nc.gpsimd.dma_start(xt[:], x_t[k])
                xtile[k] = xt

            def emit_A(k):
                pA = ppa_pool.tile([128, 2 * NP], F32, tag="pA")
                nc.tensor.matmul(pA[:], xtile.pop(k)[:], gt264[:],
                                 start=True, stop=True)
                r1 = r1pool.tile([128, 2 * NP], BF16, tag="r1")
                nc.scalar.copy(r1[:], pA[:])
                r1s[k] = r1

            def emit_B(k):
                c = k // B
                khi, klo = chan[c]
                if k % B == 0:
                    ylop = ypool.tile([4, 4 * 2 * NV], BF16, tag="ylop")
                    ylops[c] = ylop
                ylop = ylops[c]
                r1 = r1s.pop(k)
                pY = ppy_pool.tile([128, 4 * NV], F32, tag="pY")
                nc.tensor.matmul(pY[:, 0:2 * NV], r1[:, 0:128],
                                 bgt1[:], start=True, stop=False)
                nc.tensor.matmul(pY[:, 0:2 * NV], r1[:, NP:NP + 128],
                                 bgt2[:], start=False, stop=True)
                nc.tensor.matmul(pY[0:4, 2 * NV:4 * NV], r1[:, 128:NP],
                                 bgt1[:], start=True, stop=False)
                nc.tensor.matmul(pY[0:4, 2 * NV:4 * NV],
                                 r1[:, NP + 128:2 * NP],
                                 bgt2[:], start=False, stop=True)

                bi = k % B
                yall = ypool.tile([128, 2 * NV], BF16, tag="yall")
                nc.scalar.copy(yall[:], pY[:, 0:2 * NV])
                nc.scalar.copy(ylop[:, bi * 2 * NV:(bi + 1) * 2 * NV],
                               pY[0:4, 2 * NV:4 * NV])

                y_b = (yall[:]
                       .rearrange("p (a v) -> p a v", a=2)
                       [:, :, None, :]
                       .broadcast_to([128, 2, 4, NV]))
                pa = papool.tile([128, 2 * PH4], BF16, tag="pa")
                nc.vector.tensor_mul(
                    pa[:].rearrange("p (a f v) -> p a f v", a=2, f=4),
                    khi[:, 0:2 * PH4].rearrange("p (a f v) -> p a f v",
                                                a=2, f=4),
                    y_b)
                pb = papool.tile([128, 2 * PH4], BF16, tag="pb")
                nc.vector.tensor_mul(
                    pb[:].rearrange("p (a f v) -> p a f v", a=2, f=4),
                    khi[:, 2 * PH4:4 * PH4].rearrange(
                        "p (a f v) -> p a f v", a=2, f=4),
                    y_b)
                fxr = fxpool.tile([128, PH4], BF16, tag="fxr")
                nc.gpsimd.tensor_sub(fxr[:], pa[:, 0:PH4],
                                     pa[:, PH4:2 * PH4])
                fxi = fxpool.tile([128, PH4], BF16, tag="fxi")
                nc.vector.tensor_add(fxi[:], pb[:, 0:PH4],
                                     pb[:, PH4:2 * PH4])
                fxs[k] = (fxr, fxi)
                if k % B == B - 1:
                    emit_locmul(c)

            def emit_locmul(c):
                _, klo = chan[c]
                ylop = ylops.pop(c)
                ylo_b = (ylop[:]
                         .rearrange("p (ba v) -> p ba v", ba=8)
                         [:, :, None, :]
                         .broadcast_to([4, 8, 4, NV]))
                palo = papool.tile([4, 8 * PH4], BF16, tag="palo")
                nc.vector.tensor_mul(
                    palo[:].rearrange("p (ba f v) -> p ba f v", ba=8, f=4),
                    klo[:, 0:8 * PH4].rearrange("p (ba f v) -> p ba f v",
                                                ba=8, f=4),
                    ylo_b)
                pblo = papool.tile([4, 8 * PH4], BF16, tag="pblo")
                nc.vector.tensor_mul(
                    pblo[:].rearrange("p (ba f v) -> p ba f v", ba=8, f=4),
                    klo[:, 8 * PH4:16 * PH4].rearrange(
                        "p (ba f v) -> p ba f v", ba=8, f=4),
                    ylo_b)
                fxlr = fxpool.tile([4, 4 * PH4], BF16, tag="fxlr")
                nc.gpsimd.tensor_sub(
                    fxlr[:].rearrange("p (b x) -> p b x", b=4),
                    palo[:].rearrange("p (b a x) -> p b a x", b=4, a=2)
                    [:, :, 0, :],
                    palo[:].rearrange("p (b a x) -> p b a x", b=4, a=2)
                    [:, :, 1, :])
                fxli = fxpool.tile([4, 4 * PH4], BF16, tag="fxli")
                nc.vector.tensor_add(
                    fxli[:].rearrange("p (b x) -> p b x", b=4),
                    pblo[:].rearrange("p (b a x) -> p b a x", b=4, a=2)
                    [:, :, 0, :],
                    pblo[:].rearrange("p (b a x) -> p b a x", b=4, a=2)
                    [:, :, 1, :])
                fxlos[c] = (fxlr, fxli)

            t1s = {}

            def emit_C(k):
                c, bi = k // B, k % B
                fxr, fxi = fxs.pop(k)
                fxlr, fxli = fxlos[c]
                pt1 = pt1_pool.tile([NV, 1024], F32, tag="pt1")
                for p in range(4):
                    o = pt1[:, ts(p, 256)]
                    lo0 = bi * PH4 + p * NV
                    nc.tensor.matmul(o, fxr[:, ts(p, NV)], cst1[:],
                                     start=True, stop=False)
                    nc.tensor.matmul(o, fxi[:, ts(p, NV)], cst2[:],
                                     start=False, stop=False)
                    nc.tensor.matmul(o, fxlr[:, lo0:lo0 + NV],
                                     cst1lo[:], start=False, stop=False)
                    nc.tensor.matmul(o, fxli[:, lo0:lo0 + NV],
                                     cst2lo[:], start=False, stop=True)
                t1 = t1pool.tile([NV, 1024], BF16, tag="t1")
                nc.scalar.copy(t1[:, 0:448], pt1[:, 0:448])
                nc.vector.tensor_copy(t1[:, 448:1024], pt1[:, 448:1024])
                t1s[k] = t1

            def emit_D(k):
                t1 = t1s.pop(k)
                pD = ppd_pool.tile([128, 1024], F32, tag="pD")
                for p in range(4):
                    o = pD[:, ts(p, 256)]
                    nc.tensor.matmul(o, t1[:, p * 256:p * 256 + 128],
                                     rcs[:], start=True, stop=False)
                    nc.tensor.matmul(o, t1[:, p * 256 + 128:(p + 1) * 256],
                                     rcs2[:], start=False, stop=True)
                pDv = pD[:].rearrange("q (p s v) -> q p s v", p=4, s=2)
                osb = opool.tile([128, 512], F32, tag="osb")
                nc.scalar.activation(
                    osb[:, 0:256].rearrange("p (v d) -> p d v", d=2),
                    pDv[:, 0:2, 0, :],
                    AF.Gelu)
                nc.scalar.activation(
                    osb[:, 256:512].rearrange("p (v d) -> p d v", d=2),
                    pDv[:, 2:4, 0, :],
                    AF.Gelu)
                nc.sync.dma_start(
                    out_t[k].rearrange("(x d) y -> x (d y)", d=2),
                    osb[:])

            # prelude: load channel 0, stage 5 images, run A/B for the
            # first 4 so C'(0) and the channel-0 lo-cmul are unblocked
            emit_kdma(0)
            for j in range(5):
                emit_xdma(j)
            emit_A(0)
            emit_A(1)
            emit_B(0)
            emit_A(2)
            emit_B(1)
            emit_A(3)
            emit_B(2)
            emit_A(4)
            emit_B(3)

            for k in range(NIMG):
                if k + 4 < NIMG and (k + 4) % B == 0:
                    emit_kdma((k + 4) // B)
                if k + 5 < NIMG:
                    emit_xdma(k + 5)
                emit_C(k)
                if k + 5 < NIMG:
                    emit_A(k + 5)
                if k + 4 < NIMG:
                    emit_B(k + 4)
                emit_D(k)

    nc.compile()
    return nc


# --------------------------------------------------------------------------
# public entry point: full inputs in, full output out
# --------------------------------------------------------------------------

def kernel(x, weight, bias):
    global LAST_EXEC_NS
    x = np.asarray(x, dtype=np.float32)
    consts = _host_constants(weight, bias)

    nc = build_nc()

    bf = ml_dtypes.bfloat16
    in_maps = []
    for core in range(NCORES):
        c0 = core * CPC
        xs = np.ascontiguousarray(
            x[:, c0:c0 + CPC].transpose(1, 0, 2, 3)).reshape(
                NIMG, N0, N0).astype(bf)
        in_maps.append({
            "x": xs,
            "khi": np.ascontiguousarray(consts["khi"][c0:c0 + CPC]),
            "klo": np.ascontiguousarray(consts["klo"][c0:c0 + CPC]),
            "gt264": consts["gt264"],
            "bgt1": consts["bgt1"],
            "bgt2": consts["bgt2"],
            "cst1": consts["cst1"],
            "cst2": consts["cst2"],
            "cst1lo": consts["cst1lo"],
            "cst2lo": consts["cst2lo"],
            "rcs": consts["rcs"],
            "rcs2": consts["rcs2"],
        })

    trace = os.environ.get("KERNEL_TRACE", "0") == "1"
    tmpdir = os.environ.get("KERNEL_TMPDIR") or None
    res = run_bass_kernel_spmd(nc, in_maps, list(range(NCORES)), trace=trace,
                               tmpdir=tmpdir)
    LAST_EXEC_NS = res.exec_time_ns

    out = np.empty((B, C, 2 * N0, 2 * N0), dtype=np.float32)
    for core in range(NCORES):
        c0 = core * CPC
        o = res.results[core]["out"].reshape(CPC, B, 2 * N0, 2 * N0)
        out[:, c0:c0 + CPC] = o.transpose(1, 0, 2, 3)
    return out


# revision 36
# speedup vs baseline: 1.2025x; 1.2025x over previous
"""Converse2D-Up (FFT deconvolution upsampler) as a Bass/Tile kernel for TRN2.

Math (validated against the jax reference): the whole pipeline before the
final gelu is linear in x and channel-wise.  With xp = wrap-pad(x) (132x132),
Y = FFT132(xp) = G @ x @ G^T where G = F132 @ P (132x128).  The reference's
264-point spectral transfer function H (built from weight/bias only) is
Hermitian, so out decomposes into 4 polyphase outputs
out_dd = real(IFFT132(Kdd_hat . Y)) with per-channel precomputed spectra
Kdd_hat; the crop leaves exactly 128 rows/cols per phase.  Hermitian symmetry
means only columns v=0..66 of Kdd_hat.Y are needed:
    T1[v,x] = sum_u (Kdd_hat.Y)[u,pv] Ai[x,u]
    out[x,y] = sum_{v} w_v (T1r[v,x] RC[v,y] + T1i[v,x] RS[v,y])

v2: all PE matmuls in bf16 (1 cycle/row vs 4 for fp32), packed N>=256 moving
operands (29 matmuls/image vs 98), the 4 low rows (u=128..131) of the
pointwise complex multiply batched across the 4 images of a channel, and the
gelu+interleave output written with a single contiguous DMA per image.

Sharding: 8 channels per core x 4 batch images; weight/bias-derived spectra
are host-precomputed constants.
"""

import os

import ml_dtypes
import numpy as np

import concourse.bass as bass
import concourse.mybir as mybir
import concourse.tile as tile
from concourse import bacc
from concourse.bass import ts
from concourse.bass_utils import run_bass_kernel_spmd

F32 = mybir.dt.float32
BF16 = mybir.dt.bfloat16
AF = mybir.ActivationFunctionType

SCALE = 2
PAD = 2
EPS = 1e-5
N0 = 128           # input spatial size
NP = N0 + 2 * PAD  # 132 padded
NU = NP * SCALE    # 264 upsampled
NV = NP // 2 + 1   # 67 unique spectral columns
B = 4
C = 64
NCORES = 8
CPC = C // NCORES  # 8 channels per core
NIMG = B * CPC     # 32 images per core

LAST_EXEC_NS = None  # set by kernel() when tracing is enabled


# --------------------------------------------------------------------------
# host-side constant precompute (weight/bias -> per-channel spectra)
# --------------------------------------------------------------------------

def _host_constants(weight, bias):
    w64 = np.asarray(weight, dtype=np.float64)
    b64 = np.asarray(bias, dtype=np.float64)

    # FB = p2o(weight): 264-point OTF of the rolled 3x3 PSF, per channel
    k_h, k_w = w64.shape[-2:]
    otf = np.zeros((C, NU, NU), dtype=np.complex128)
    otf[:, :k_h, :k_w] = w64[0]
    otf = np.roll(otf, (-(k_h // 2), -(k_w // 2)), axis=(-2, -1))
    FB = np.fft.fftn(otf, axes=(-2, -1))                      # (C,264,264)

    biaseps = 1.0 / (1.0 + np.exp(-(b64.reshape(C) - 9.0))) + EPS  # (C,)
    be = biaseps[:, None, None]

    u = np.arange(NU)
    Dr = 1 + np.exp(-2j * np.pi * u / NU)
    D = Dr[:, None] * Dr[None, :]                             # (264,264)

    Gh = np.conj(FB) + be * D[None]
    FBG = FB * Gh

    def quadmean(A):
        return 0.25 * (A[:, :NP, :NP] + A[:, NP:, :NP]
                       + A[:, :NP, NP:] + A[:, NP:, NP:])

    M1 = quadmean(FBG)
    invW = quadmean(np.abs(FB) ** 2)
    M2 = M1 / (invW + be)
    H = (Gh - np.conj(FB) * np.tile(M2, (1, SCALE, SCALE))) / be   # (C,264,264)

    hr = np.fft.ifft2(H, axes=(-2, -1)).real                  # H Hermitian
    # polyphase spectra: Kdd_hat[c,dx,dy] = FFT132(hr[c, dx::2, dy::2])
    kdd = np.empty((C, 2, 2, NP, NV), dtype=np.complex128)
    for dx in range(2):
        for dy in range(2):
            kh = np.fft.fft2(hr[:, dx::2, dy::2], axes=(-2, -1))
            kdd[:, dx, dy] = kh[:, :, :NV]

    # forward matrix G = F132 @ P  (132x128 complex)
    P = np.zeros((NP, N0))
    for m in range(NP):
        P[m, (m - PAD) % N0] = 1.0
    F132 = np.exp(-2j * np.pi * np.outer(np.arange(NP), np.arange(NP)) / NP)
    G = F132 @ P

    # inverse matrix, rows i in [2,130) of iF132/132
    Ai = np.exp(2j * np.pi * np.outer(np.arange(2, 130), np.arange(NP)) / NP) / NP
    Cm, Sm = Ai.real, Ai.imag                                  # (128,132)
    CT, ST = Cm.T, Sm.T                                        # (132,128)

    w_v = np.ones(NV)
    w_v[1:NV - 1] = 2.0
    RC = (Cm[:, :NV] * w_v[None, :]).T                         # (67,128)
    RS = (-Sm[:, :NV] * w_v[None, :]).T

    bf = ml_dtypes.bfloat16

    # per-channel spectra packed for the DVE complex multiply:
    # cols (a, p, v): a in {0,1}; set1=(Kr,Ki), set2=(Ki,Kr)
    kr = np.ascontiguousarray(
        kdd.real.transpose(0, 3, 1, 2, 4)).reshape(C, NP, 4 * NV)
    ki = np.ascontiguousarray(
        kdd.imag.transpose(0, 3, 1, 2, 4)).reshape(C, NP, 4 * NV)
    khi = np.concatenate([kr[:, :128], ki[:, :128],
                          ki[:, :128], kr[:, :128]], axis=2)   # (C,128,1072)
    # lo rows (u=128..131) at partition strips {0,32,64,96}, one strip per
    # batch image: DVE cost scales with free size only, so the 4-image cmul
    # stays as cheap as a single image's; the C' stationary slices then use
    # PE tile_position for the strip offsets
    klo_s = np.concatenate([kr[:, 128:], ki[:, 128:],
                            ki[:, 128:], kr[:, 128:]], axis=2)   # (C,4,1072)
    klo = np.zeros((C, 100, 1072), klo_s.dtype)
    for b in range(4):
        klo[:, 32 * b:32 * b + 4] = klo_s

    return {
        "gt264": np.concatenate([G.real.T, G.imag.T], 1).astype(bf),
        "bgt1": np.concatenate([G.real[0:NV].T, G.imag[0:NV].T], 1).astype(bf),
        "bgt2": np.concatenate([-G.imag[0:NV].T, G.real[0:NV].T], 1).astype(bf),
        "cst1": np.concatenate([CT[:128], ST[:128]], 1).astype(bf),
        "cst2": np.concatenate([-ST[:128], CT[:128]], 1).astype(bf),
        "cst1lo": np.tile(np.concatenate([CT[128:], ST[128:]], 1),
                          (25, 1))[:100].astype(bf),
        "cst2lo": np.tile(np.concatenate([-ST[128:], CT[128:]], 1),
                          (25, 1))[:100].astype(bf),
        "rcs": np.concatenate([RC, RS], 1).astype(bf),
        "rcs2": np.concatenate([RS, RC], 1).astype(bf),
        "khi": khi.astype(bf),
        "klo": klo.astype(bf),
    }


# --------------------------------------------------------------------------
# device kernel
# --------------------------------------------------------------------------

def build_nc():
    nc = bacc.Bacc("TRN2", target_bir_lowering=False, debug=False,
                   enable_asserts=False)

    x_t = nc.dram_tensor("x", [NIMG, N0, N0], BF16, kind="ExternalInput")
    khi_t = nc.dram_tensor("khi", [CPC, 128, 1072], BF16, kind="ExternalInput")
    klo_t = nc.dram_tensor("klo", [CPC, 100, 1072], BF16, kind="ExternalInput")
    gt264_t = nc.dram_tensor("gt264", [128, 2 * NP], BF16, kind="ExternalInput")
    bgt1_t = nc.dram_tensor("bgt1", [128, 2 * NV], BF16, kind="ExternalInput")
    bgt2_t = nc.dram_tensor("bgt2", [128, 2 * NV], BF16, kind="ExternalInput")
    cst1_t = nc.dram_tensor("cst1", [128, 256], BF16, kind="ExternalInput")
    cst2_t = nc.dram_tensor("cst2", [128, 256], BF16, kind="ExternalInput")
    cst1lo_t = nc.dram_tensor("cst1lo", [100, 256], BF16, kind="ExternalInput")
    cst2lo_t = nc.dram_tensor("cst2lo", [100, 256], BF16, kind="ExternalInput")
    rcs_t = nc.dram_tensor("rcs", [NV, 256], BF16, kind="ExternalInput")
    rcs2_t = nc.dram_tensor("rcs2", [NV, 256], BF16, kind="ExternalInput")
    out_t = nc.dram_tensor("out", [NIMG, 2 * N0, 2 * N0], F32,
                           kind="ExternalOutput")

    PH4 = 4 * NV          # 268
    with tile.TileContext(nc) as tc:
        with (
            tc.tile_pool(name="consts", bufs=1) as cpool,
            tc.tile_pool(name="kdd", bufs=2) as kpool,
            tc.tile_pool(name="xin", bufs=5) as xpool,
            tc.tile_pool(name="r1", bufs=2) as r1pool,
            tc.tile_pool(name="yev", bufs=2) as ypool,
            tc.tile_pool(name="prod", bufs=2) as papool,
            tc.tile_pool(name="fx", bufs=6) as fxpool,
            tc.tile_pool(name="t1", bufs=2) as t1pool,
            tc.tile_pool(name="osb", bufs=2) as opool,
            tc.tile_pool(name="ppa", bufs=1, space="PSUM") as ppa_pool,
            tc.tile_pool(name="ppy", bufs=1, space="PSUM") as ppy_pool,
            tc.tile_pool(name="pt1", bufs=2, space="PSUM") as pt1_pool,
            tc.tile_pool(name="ppd", bufs=1, space="PSUM") as ppd_pool,
        ):
            gt264 = cpool.tile([128, 2 * NP], BF16)
            nc.sync.dma_start(gt264[:], gt264_t[:])
            bgt1 = cpool.tile([128, 2 * NV], BF16)
            nc.sync.dma_start(bgt1[:], bgt1_t[:])
            bgt2 = cpool.tile([128, 2 * NV], BF16)
            nc.sync.dma_start(bgt2[:], bgt2_t[:])
            cst1 = cpool.tile([128, 256], BF16)
            nc.sync.dma_start(cst1[:], cst1_t[:])
            cst2 = cpool.tile([128, 256], BF16)
            nc.sync.dma_start(cst2[:], cst2_t[:])
            cst1lo = cpool.tile([100, 256], BF16)
            nc.sync.dma_start(cst1lo[:], cst1lo_t[:])
            cst2lo = cpool.tile([100, 256], BF16)
            nc.sync.dma_start(cst2lo[:], cst2lo_t[:])
            rcs = cpool.tile([NV, 256], BF16)
            nc.sync.dma_start(rcs[:], rcs_t[:])
            rcs2 = cpool.tile([NV, 256], BF16)
            nc.sync.dma_start(rcs2[:], rcs2_t[:])

            # -------- software-pipelined slot schedule ----------------
            # slot k emits: C'(k), A(k+5), B(k+4), D(k) on the PE so the
            # psum evictions of image k (scalar/DVE) hide under A/B, and
            # r1(k+5) is ready one full slot before B(k+5) consumes it.
            chan = {}     # c -> (khi, klo)
            xtile = {}    # k -> x tile
            r1s = {}      # k -> r1 tile
            ylops = {}    # c -> ylop tile
            fxs = {}      # k -> (fxr, fxi)
            fxlos = {}    # c -> (fxlr, fxli)

            def emit_kdma(c):
                khi = kpool.tile([128, 1072], BF16, tag="khi")
                nc.sync.dma_start(khi[:], khi_t[c])
                klo = kpool.tile([100, 1072], BF16, tag="klo")
                nc.sync.dma_start(klo[:], klo_t[c])
                chan[c] = (khi, klo)

            def emit_xdma(k):
                xt = xpool.tile([N0, N0], BF16, tag="x")
                nc.gpsimd.dma_start(xt[:], x_t[k])
                xtile[k] = xt

            def emit_A(k):
                pA = ppa_pool.tile([128, 2 * NP], F32, tag="pA")
                nc.tensor.matmul(pA[:], xtile.pop(k)[:], gt264[:],
                                 start=True, stop=True)
                r1 = r1pool.tile([128, 2 * NP], BF16, tag="r1")
                nc.scalar.copy(r1[:], pA[:])
                r1s[k] = r1

            def emit_B(k):
                c = k // B
                khi, klo = chan[c]
                if k % B == 0:
                    ylop = ypool.tile([100, 2 * NV], BF16, tag="ylop")
                    ylops[c] = ylop
                ylop = ylops[c]
                r1 = r1s.pop(k)
                pY = ppy_pool.tile([128, 4 * NV], F32, tag="pY")
                nc.tensor.matmul(pY[:, 0:2 * NV], r1[:, 0:128],
                                 bgt1[:], start=True, stop=False)
                nc.tensor.matmul(pY[:, 0:2 * NV], r1[:, NP:NP + 128],
                                 bgt2[:], start=False, stop=True)
                nc.tensor.matmul(pY[0:4, 2 * NV:4 * NV], r1[:, 128:NP],
                                 bgt1[:], start=True, stop=False)
                nc.tensor.matmul(pY[0:4, 2 * NV:4 * NV],
                                 r1[:, NP + 128:2 * NP],
                                 bgt2[:], start=False, stop=True)

                bi = k % B
                yall = ypool.tile([128, 2 * NV], BF16, tag="yall")
                nc.scalar.copy(yall[:], pY[:, 0:2 * NV])
                nc.scalar.copy(ylop[32 * bi:32 * bi + 4, :],
                               pY[0:4, 2 * NV:4 * NV])

                y_b = (yall[:]
                       .rearrange("p (a v) -> p a v", a=2)
                       [:, :, None, :]
                       .broadcast_to([128, 2, 4, NV]))
                pa = papool.tile([128, 2 * PH4], BF16, tag="pa")
                nc.vector.tensor_mul(
                    pa[:].rearrange("p (a f v) -> p a f v", a=2, f=4),
                    khi[:, 0:2 * PH4].rearrange("p (a f v) -> p a f v",
                                                a=2, f=4),
                    y_b)
                pb = papool.tile([128, 2 * PH4], BF16, tag="pb")
                nc.vector.tensor_mul(
                    pb[:].rearrange("p (a f v) -> p a f v", a=2, f=4),
                    khi[:, 2 * PH4:4 * PH4].rearrange(
                        "p (a f v) -> p a f v", a=2, f=4),
                    y_b)
                fxr = fxpool.tile([128, PH4], BF16, tag="fxr")
                nc.gpsimd.tensor_sub(fxr[:], pa[:, 0:PH4],
                                     pa[:, PH4:2 * PH4])
                fxi = fxpool.tile([128, PH4], BF16, tag="fxi")
                nc.vector.tensor_add(fxi[:], pb[:, 0:PH4],
                                     pb[:, PH4:2 * PH4])
                fxs[k] = (fxr, fxi)
                if k % B == B - 1:
                    emit_locmul(c)

            def emit_locmul(c):
                _, klo = chan[c]
                ylop = ylops.pop(c)
                ylo_b = (ylop[:]
                         .rearrange("p (a v) -> p a v", a=2)
                         [:, :, None, :]
                         .broadcast_to([100, 2, 4, NV]))
                palo = papool.tile([100, 2 * PH4], BF16, tag="palo")
                nc.vector.tensor_mul(
                    palo[:].rearrange("p (a f v) -> p a f v", a=2, f=4),
                    klo[:, 0:2 * PH4].rearrange("p (a f v) -> p a f v",
                                                a=2, f=4),
                    ylo_b)
                pblo = papool.tile([100, 2 * PH4], BF16, tag="pblo")
                nc.vector.tensor_mul(
                    pblo[:].rearrange("p (a f v) -> p a f v", a=2, f=4),
                    klo[:, 2 * PH4:4 * PH4].rearrange("p (a f v) -> p a f v",
                                                      a=2, f=4),
                    ylo_b)
                fxlr = fxpool.tile([100, PH4], BF16, tag="fxlr")
                nc.gpsimd.tensor_sub(fxlr[:], palo[:, 0:PH4],
                                     palo[:, PH4:2 * PH4])
                fxli = fxpool.tile([100, PH4], BF16, tag="fxli")
                nc.vector.tensor_add(fxli[:], pblo[:, 0:PH4],
                                     pblo[:, PH4:2 * PH4])
                fxlos[c] = (fxlr, fxli)

            t1s = {}

            def emit_C(k):
                c, bi = k // B, k % B
                fxr, fxi = fxs.pop(k)
                fxlr, fxli = fxlos[c]
                pt1 = pt1_pool.tile([NV, 1024], F32, tag="pt1")
                s0 = 32 * bi
                for p in range(4):
                    o = pt1[:, ts(p, 256)]
                    nc.tensor.matmul(o, fxr[:, ts(p, NV)], cst1[:],
                                     start=True, stop=False)
                    nc.tensor.matmul(o, fxi[:, ts(p, NV)], cst2[:],
                                     start=False, stop=False)
                    nc.tensor.matmul(o, fxlr[s0:s0 + 4, ts(p, NV)],
                                     cst1lo[s0:s0 + 4, :],
                                     start=False, stop=False,
                                     tile_position=(s0, 0))
                    nc.tensor.matmul(o, fxli[s0:s0 + 4, ts(p, NV)],
                                     cst2lo[s0:s0 + 4, :],
                                     start=False, stop=True,
                                     tile_position=(s0, 0))
                t1 = t1pool.tile([NV, 1024], BF16, tag="t1")
                nc.scalar.copy(t1[:, 0:448], pt1[:, 0:448])
                nc.vector.tensor_copy(t1[:, 448:1024], pt1[:, 448:1024])
                t1s[k] = t1

            def emit_D(k):
                t1 = t1s.pop(k)
                pD = ppd_pool.tile([128, 512], F32, tag="pD")
                for p in range(4):
                    o = pD[:, ts(p, 128)]
                    nc.tensor.matmul(o, t1[:, p * 256:p * 256 + 128],
                                     rcs[:, 0:128], start=True, stop=False)
                    nc.tensor.matmul(o, t1[:, p * 256 + 128:(p + 1) * 256],
                                     rcs[:, 128:256], start=False, stop=True)
                osb = opool.tile([128, 512], F32, tag="osb")
                nc.scalar.activation(
                    osb[:, 0:256].rearrange("p (v d) -> p d v", d=2),
                    pD[:, 0:256].rearrange("p (d v) -> p d v", d=2),
                    AF.Gelu)
                nc.scalar.activation(
                    osb[:, 256:512].rearrange("p (v d) -> p d v", d=2),
                    pD[:, 256:512].rearrange("p (d v) -> p d v", d=2),
                    AF.Gelu)
                nc.sync.dma_start(
                    out_t[k].rearrange("(x d) y -> x (d y)", d=2),
                    osb[:])

            # prelude: load channel 0, stage 5 images, run A/B for the
            # first 4 so C'(0) and the channel-0 lo-cmul are unblocked
            emit_kdma(0)
            for j in range(5):
                emit_xdma(j)
            emit_A(0)
            emit_A(1)
            emit_B(0)
            emit_A(2)
            emit_B(1)
            emit_A(3)
            emit_B(2)
            emit_A(4)
            emit_B(3)

            for k in range(NIMG):
                if k + 4 < NIMG and (k + 4) % B == 0:
                    emit_kdma((k + 4) // B)
                if k + 5 < NIMG:
                    emit_xdma(k + 5)
                emit_C(k)
                if k + 5 < NIMG:
                    emit_A(k + 5)
                if k + 4 < NIMG:
                    emit_B(k + 4)
                emit_D(k)

    nc.compile()
    return nc


# --------------------------------------------------------------------------
# public entry point: full inputs in, full output out
# --------------------------------------------------------------------------

def kernel(x, weight, bias):
    global LAST_EXEC_NS
    x = np.asarray(x, dtype=np.float32)
    consts = _host_constants(weight, bias)

    nc = build_nc()

    bf = ml_dtypes.bfloat16
    in_maps = []
    for core in range(NCORES):
        c0 = core * CPC
        xs = np.ascontiguousarray(
            x[:, c0:c0 + CPC].transpose(1, 0, 2, 3)).reshape(
                NIMG, N0, N0).astype(bf)
        in_maps.append({
            "x": xs,
            "khi": np.ascontiguousarray(consts["khi"][c0:c0 + CPC]),
            "klo": np.ascontiguousarray(consts["klo"][c0:c0 + CPC]),
            "gt264": consts["gt264"],
            "bgt1": consts["bgt1"],
            "bgt2": consts["bgt2"],
            "cst1": consts["cst1"],
            "cst2": consts["cst2"],
            "cst1lo": consts["cst1lo"],
            "cst2lo": consts["cst2lo"],
            "rcs": consts["rcs"],
            "rcs2": consts["rcs2"],
        })

    trace = os.environ.get("KERNEL_TRACE", "0") == "1"
    tmpdir = os.environ.get("KERNEL_TMPDIR") or None
    res = run_bass_kernel_spmd(nc, in_maps, list(range(NCORES)), trace=trace,
                               tmpdir=tmpdir)
    LAST_EXEC_NS = res.exec_time_ns

    out = np.empty((B, C, 2 * N0, 2 * N0), dtype=np.float32)
    for core in range(NCORES):
        c0 = core * CPC
        o = res.results[core]["out"].reshape(CPC, B, 2 * N0, 2 * N0)
        out[:, c0:c0 + CPC] = o.transpose(1, 0, 2, 3)
    return out
